# revision 2
# baseline (speedup 1.0000x reference)
"""Trainium2 kernel for nn_ClusteringLayer (vq_codebook).

Problem: x (1, 131072, 256) f32, cluster_centers (1024, 256) f32.
For each cluster k: find argmin_n ||x[n] - c[k]||^2 and return that x row.
Output: (1, 1024, 256) f32.

Strategy (8 NeuronCores, x sharded along n, centers replicated):
  argmin_n d2[n,k] == argmax_n s[n,k],  s = 2*x.c - |x|^2  (c2[k] const per k)
  Device per core:
    psum[k_tile, n_grp] = bf16 matmul of [xT ; x2_hi ; x2_lo] against
    [(2C)T ; -1 ; -1]  -> psum = 2*dot - x2 directly (x2 split into two
    bf16 rows keeps the subtraction accurate to ~0.01).
    VectorE reduce_max over each 2048-point group -> bmax[k, grp] f32.
  Host:
    winner group per cluster + all groups within THETA of the winner get
    exact fp32/fp64 rescoring; argmin with first-index tiebreak matches
    the fp32 reference (validated on the real input: worst true-group
    margin 0.30 vs THETA=3.0).
"""

import os
import sys

for _p in ("/opt/trn_rl_repo",):
    if os.path.isdir(_p) and _p not in sys.path:
        sys.path.append(_p)

import numpy as np
import ml_dtypes

import concourse.bass as bass
import concourse.bacc as bacc
import concourse.mybir as mybir
import concourse.tile as tile

NCORES = 8
N = 131072
F = 256
K = 1024
SH = N // NCORES            # 16384 points per core
GRP = 2048                  # group size for the device-side max reduction
NG = SH // GRP              # 8 groups per core
KT = K // 128               # 8 cluster tiles
NCH = F // 128              # 2 contraction chunks
THETA = 3.0                 # host rescue radius on group maxima
TOPM = 32                   # fp32->fp64 refine width per (cluster, group)

BF16 = ml_dtypes.bfloat16


def build_nc():
    """Build + compile the per-core Bass program (same program on all cores)."""
    nc = bacc.Bacc("TRN2", target_bir_lowering=False, debug=False,
                   num_devices=NCORES)

    xt = nc.dram_tensor("xt", [NCH, 128, SH], mybir.dt.bfloat16,
                        kind="ExternalInput")
    x2r = nc.dram_tensor("x2r", [2, SH], mybir.dt.bfloat16,
                         kind="ExternalInput")
    ct2 = nc.dram_tensor("ct2", [NCH, 128, K], mybir.dt.bfloat16,
                         kind="ExternalInput")
    bmax_d = nc.dram_tensor("bmax", [128, KT * NG], mybir.dt.float32,
                            kind="ExternalOutput")

    with tile.TileContext(nc) as tc:
        with (
            tc.tile_pool(name="consts", bufs=1) as cpool,
            tc.tile_pool(name="xtp", bufs=3) as xpool,
            tc.tile_pool(name="psum", bufs=2, space="PSUM") as ppool,
        ):
            ct2_t = []
            for ch in range(NCH):
                t = cpool.tile([128, K], mybir.dt.bfloat16, tag=f"ct{ch}")
                nc.sync.dma_start(t[:], ct2[ch, :, :])
                ct2_t.append(t)
            x2r_t = cpool.tile([2, SH], mybir.dt.bfloat16, tag="x2r")
            nc.sync.dma_start(x2r_t[:], x2r[:, :])
            neg1_t = cpool.tile([2, 128], mybir.dt.bfloat16, tag="neg1")
            nc.gpsimd.memset(neg1_t[:], -1.0)
            bmax_t = cpool.tile([128, KT * NG], mybir.dt.float32, tag="bmax")

            for g in range(NG):
                xg = []
                for ch in range(NCH):
                    t = xpool.tile([128, GRP], mybir.dt.bfloat16, tag=f"xt{ch}")
                    nc.sync.dma_start(t[:], xt[ch, :, g * GRP:(g + 1) * GRP])
                    xg.append(t)

                for kt in range(KT):
                    ps = ppool.tile([128, GRP], mybir.dt.float32, tag="ps")
                    for ch in range(NCH):
                        for blk in range(GRP // 512):
                            nc.tensor.matmul(
                                ps[:, blk * 512:(blk + 1) * 512],
                                lhsT=ct2_t[ch][:, kt * 128:(kt + 1) * 128],
                                rhs=xg[ch][:, blk * 512:(blk + 1) * 512],
                                start=(ch == 0),
                                stop=False,
                            )
                    for blk in range(GRP // 512):
                        nc.tensor.matmul(
                            ps[:, blk * 512:(blk + 1) * 512],
                            lhsT=neg1_t[:, :],
                            rhs=x2r_t[:, g * GRP + blk * 512:
                                      g * GRP + (blk + 1) * 512],
                            start=False,
                            stop=True,
                        )
                    col = kt * NG + g
                    nc.vector.tensor_reduce(
                        out=bmax_t[:, col:col + 1],
                        in_=ps[:],
                        axis=mybir.AxisListType.X,
                        op=mybir.AluOpType.max,
                    )

            nc.sync.dma_start(bmax_d[:, :], bmax_t[:])

    nc.compile()
    return nc


def host_inputs(x, cluster_centers):
    """Per-core input dicts for run_bass_kernel_spmd."""
    x0 = np.ascontiguousarray(x[0], dtype=np.float32)        # (N, F)
    C = np.ascontiguousarray(cluster_centers, dtype=np.float32)
    x2 = np.einsum('nf,nf->n', x0.astype(np.float64),
                   x0.astype(np.float64)).astype(np.float32)
    x2_hi = x2.astype(BF16)
    x2_lo = (x2 - x2_hi.astype(np.float32)).astype(BF16)
    ct2_np = np.ascontiguousarray(
        (2.0 * C).T.astype(BF16)).reshape(NCH, 128, K)
    in_maps = []
    for c in range(NCORES):
        sl = slice(c * SH, (c + 1) * SH)
        xs = x0[sl]
        xt_np = np.ascontiguousarray(xs.T.astype(BF16)).reshape(NCH, 128, SH)
        x2r_np = np.ascontiguousarray(np.stack([x2_hi[sl], x2_lo[sl]]))
        in_maps.append({"xt": xt_np, "x2r": x2r_np, "ct2": ct2_np})
    return in_maps


def host_combine(bmax_cores, x, cluster_centers):
    """Exact argmin recovery from per-core per-group maxima."""
    x0 = np.ascontiguousarray(x[0], dtype=np.float32)
    C = np.ascontiguousarray(cluster_centers, dtype=np.float32)
    x64 = x0.astype(np.float64)
    C64 = C.astype(np.float64)
    x2_64 = np.einsum('nf,nf->n', x64, x64)
    x2_32 = x2_64.astype(np.float32)

    # bmax_cores[c]: [128, KT*NG] -> cluster k = kt*128 + p, col = kt*NG + g
    bm = np.empty((K, NCORES * NG), dtype=np.float32)
    for c in range(NCORES):
        a = np.asarray(bmax_cores[c]).reshape(128, KT, NG)
        bm[:, c * NG:(c + 1) * NG] = a.transpose(1, 0, 2).reshape(K, NG)

    winval = bm.max(axis=1)
    flags = bm >= (winval[:, None] - THETA)     # (K, 64)

    pair_clusters = [[] for _ in range(NCORES * NG)]
    ks_idx, ps_idx = np.nonzero(flags)
    for kk, p in zip(ks_idx, ps_idx):
        pair_clusters[p].append(kk)

    best_val = np.full(K, np.inf)
    best_idx = np.zeros(K, dtype=np.int64)
    for p, ks in enumerate(pair_clusters):
        if not ks:
            continue
        c, g = divmod(p, NG)
        base = c * SH + g * GRP
        pts = x0[base:base + GRP]
        d32 = x2_32[base:base + GRP, None] - 2.0 * (pts @ C[ks].T)
        part = np.argpartition(d32, TOPM, axis=0)[:TOPM]
        for j, kk in enumerate(ks):
            ids = base + part[:, j]
            dv = x2_64[ids] - 2.0 * (x64[ids] @ C64[kk])
            o = np.lexsort((ids, dv))[0]
            if (dv[o] < best_val[kk]) or (dv[o] == best_val[kk]
                                          and ids[o] < best_idx[kk]):
                best_val[kk] = dv[o]
                best_idx[kk] = ids[o]

    return x0[best_idx][None].astype(np.float32)


_NC_CACHE = {}


def kernel(x, cluster_centers):
    from concourse.bass_utils import run_bass_kernel_spmd

    if "nc" not in _NC_CACHE:
        _NC_CACHE["nc"] = build_nc()
    nc = _NC_CACHE["nc"]

    in_maps = host_inputs(x, cluster_centers)
    res = run_bass_kernel_spmd(nc, in_maps, list(range(NCORES)))
    bmax_cores = [res.results[c]["bmax"] for c in range(NCORES)]
    return host_combine(bmax_cores, x, cluster_centers)
